# Initial kernel scaffold
#
"""Tensor-parallel compressed-linear (fp16 weights, fp32 IO) for 8 trn2 cores.

out[8, 11008] = x[8, 4096] @ W.T + bias    (W stored fp16, math in fp32)

Strategy (per spec sharding hint): shard W rows (out_features) across the 8
cores, replicate x, keep per-core output sharded along the feature dim and
concatenate on the host.

Per-core kernel: out_c[8, 1376] = x @ W_c.T + bias_c, memory-bound on the
11.27 MB fp16 weight shard (~31.5 us roofline at ~358 GB/s per-core HBM BW).

Device-side design:
  - The PE matmul contracts over the partition dim, so weights are fed as
    W.T tiles [k=128 partitions, n free]. We pre-transpose W on the host
    into a partition-major layout so every weight DMA is large and fully
    contiguous (no on-chip transpose, no strided descriptors).
  - x is fp32 but the PE streams fp16. We split x = x_hi + x_lo (two fp16
    halves) and make them 16 columns of the stationary operand: one PE pass
    over the weight stream computes both, a single cheap DVE add recombines
    them -> ~fp32 accuracy at zero extra weight traffic.
  - bias is folded into the PSUM accumulation via a K=2 matmul with
    (bias_hi, bias_lo) fp16 rows, so the epilogue is one DVE add + DMA out.
"""

import numpy as np

NCORES = 8
IN_F = 4096
OUT_F = 11008
BATCH = 8
SHARD = OUT_F // NCORES          # 1376 output features per core
P = 128
KT = IN_F // P                   # 32 k-tiles of 128
CHUNKS = 8                       # weight DMA chunks (4 k-tiles = 1.41 MB each)
KT_PER_CHUNK = KT // CHUNKS
M = 2 * BATCH                    # 16 stationary columns: [x_hi | x_lo]
# n-slices within one k-tile's 1376-wide strip (PSUM bank = 512 fp32)
N_SLICES = [(0, 512), (512, 512), (1024, 352)]

_CACHED_NC = None


def _build_bass():
    import concourse.bacc as bacc
    import concourse.mybir as mybir
    import concourse.tile as tile

    nc = bacc.Bacc("TRN2", target_bir_lowering=False, debug=False)

    # Host-pretiled weight shard: wt[p, t*SHARD + n] = W_c[n, t*128 + p]
    wt = nc.dram_tensor("wt", [P, KT * SHARD], mybir.dt.float16, kind="ExternalInput")
    # Host-pretiled x (hi/lo split): xt[p, t*M + m] = xw[t*128 + p, m]
    xt = nc.dram_tensor("xt", [P, KT * M], mybir.dt.float16, kind="ExternalInput")
    # bias hi/lo rows: b2[0, n] = bias_hi, b2[1, n] = bias_lo
    b2 = nc.dram_tensor("b2", [2, SHARD], mybir.dt.float16, kind="ExternalInput")
    out = nc.dram_tensor("out", [BATCH, SHARD], mybir.dt.float32, kind="ExternalOutput")

    with tile.TileContext(nc) as tc:
        with (
            tc.tile_pool(name="consts", bufs=1) as cpool,
            tc.tile_pool(name="wchunks", bufs=CHUNKS) as wpool,
            tc.tile_pool(name="acc", bufs=1, space="PSUM") as ppool,
            tc.tile_pool(name="outp", bufs=1) as opool,
        ):
            xt_sb = cpool.tile([P, KT * M], mybir.dt.float16)
            nc.sync.dma_start(out=xt_sb[:], in_=xt[:])
            b2_sb = cpool.tile([2, SHARD], mybir.dt.float16)
            nc.sync.dma_start(out=b2_sb[:], in_=b2[:])
            # ones[k, m] = 1 for m < BATCH else 0: adds (bias_hi + bias_lo)
            # into the hi half of the accumulator only.
            ones_sb = cpool.tile([2, M], mybir.dt.float16)
            nc.any.memset(ones_sb[:, 0:BATCH], 1.0)
            nc.any.memset(ones_sb[:, BATCH:M], 0.0)

            psum = ppool.tile([M, SHARD], mybir.dt.float32)
            for n0, nsz in N_SLICES:
                nc.tensor.matmul(
                    psum[:, n0 : n0 + nsz],
                    ones_sb[:],
                    b2_sb[:, n0 : n0 + nsz],
                    start=True,
                    stop=False,
                )

            for c in range(CHUNKS):
                chunk = wpool.tile([P, KT_PER_CHUNK * SHARD], mybir.dt.float16)
                lo = c * KT_PER_CHUNK * SHARD
                hi = (c + 1) * KT_PER_CHUNK * SHARD
                nc.sync.dma_start(out=chunk[:], in_=wt[:, lo:hi])
                for t in range(KT_PER_CHUNK):
                    ktile = c * KT_PER_CHUNK + t
                    last = ktile == KT - 1
                    lhsT = xt_sb[:, ktile * M : (ktile + 1) * M]
                    for n0, nsz in N_SLICES:
                        nc.tensor.matmul(
                            psum[:, n0 : n0 + nsz],
                            lhsT,
                            chunk[:, t * SHARD + n0 : t * SHARD + n0 + nsz],
                            start=False,
                            stop=last,
                        )

            out_sb = opool.tile([BATCH, SHARD], mybir.dt.float32)
            for n0, nsz in N_SLICES:
                nc.vector.tensor_add(
                    out=out_sb[:, n0 : n0 + nsz],
                    in0=psum[0:BATCH, n0 : n0 + nsz],
                    in1=psum[BATCH:M, n0 : n0 + nsz],
                )
                nc.sync.dma_start(out=out[:, n0 : n0 + nsz], in_=out_sb[:, n0 : n0 + nsz])

    nc.compile()
    return nc


def _get_nc():
    global _CACHED_NC
    if _CACHED_NC is None:
        _CACHED_NC = _build_bass()
    return _CACHED_NC


def _prepare_inputs(x, weight_fp16, bias):
    x32 = np.asarray(x, dtype=np.float32)
    x_hi = x32.astype(np.float16)
    x_lo = (x32 - x_hi.astype(np.float32)).astype(np.float16)
    xw = np.concatenate([x_hi.T, x_lo.T], axis=1)  # [IN_F, 16]
    xt = np.ascontiguousarray(
        xw.reshape(KT, P, M).transpose(1, 0, 2)
    ).reshape(P, KT * M)

    w = np.asarray(weight_fp16)
    assert w.dtype == np.float16 and w.shape == (OUT_F, IN_F)
    # wt_all[c, p, t*SHARD + n] = W[c*SHARD + n, t*128 + p]
    wr = w.reshape(NCORES, SHARD, KT, P)
    wt_all = np.ascontiguousarray(wr.transpose(0, 3, 2, 1)).reshape(
        NCORES, P, KT * SHARD
    )

    b32 = np.asarray(bias, dtype=np.float32)
    b_hi = b32.astype(np.float16)
    b_lo = (b32 - b_hi.astype(np.float32)).astype(np.float16)

    in_maps = []
    for c in range(NCORES):
        in_maps.append(
            {
                "wt": wt_all[c],
                "xt": xt,
                "b2": np.stack(
                    [b_hi[c * SHARD : (c + 1) * SHARD], b_lo[c * SHARD : (c + 1) * SHARD]]
                ),
            }
        )
    return in_maps


def _run(in_maps, **kwargs):
    from concourse.bass_utils import run_bass_kernel_spmd

    return run_bass_kernel_spmd(_get_nc(), in_maps, core_ids=list(range(NCORES)), **kwargs)


def kernel(x, weight_fp16, bias):
    res = _run(_prepare_inputs(x, weight_fp16, bias))
    out = np.concatenate([res.results[c]["out"] for c in range(NCORES)], axis=1)
    return np.ascontiguousarray(out, dtype=np.float32)


# revision 15
# speedup vs baseline: 1.0130x; 1.0130x over previous
"""Tensor-parallel compressed-linear (fp16 weights, fp32 IO) for 8 trn2 cores.

out[8, 11008] = x[8, 4096] @ W.T + bias    (W stored fp16, math in fp32)

Strategy (per spec sharding hint): shard W rows (out_features) across the 8
cores, replicate x, keep per-core output sharded along the feature dim and
concatenate on the host.

Per-core kernel: out_c[8, 1376] = x @ W_c.T + bias_c, memory-bound on the
11.27 MB fp16 weight shard (~31.5 us roofline at ~358 GB/s per-core HBM BW).

Device-side design:
  - The PE matmul contracts over the partition dim, so weights are fed as
    W.T tiles [k=128 partitions, n free]. We pre-transpose W on the host
    into a partition-major layout so every weight DMA is large and fully
    contiguous (no on-chip transpose, no strided descriptors).
  - x is fp32 but the PE streams fp16. We split x = x_hi + x_lo (two fp16
    halves) as columns of the stationary operand: one PE pass over the
    weight stream computes both, a cheap DVE add recombines them ->
    ~fp32 accuracy at zero extra weight traffic. The lo half sits at
    column/partition 32 because compute-engine operand partition offsets
    must be 32-aligned.
  - bias is folded into the PSUM accumulation via a K=2 matmul with
    (bias_hi, bias_lo) fp16 rows.
  - Weights stream n-major in 3 column-chunks (512/512/352 wide). Each
    chunk covers all of k, so its accumulator closes while the next chunk
    is still streaming and its epilogue (PSUM combine + store) hides under
    the stream; only the last, narrowest chunk's epilogue is exposed.
  - Each chunk streams as ~256-352 KB DMAs (k-pairs/quads): big enough
    that descriptor generation (~0.6 us/DMA) stays ahead of the wire,
    small enough that the PE tracks the stream closely.
"""

import numpy as np

NCORES = 8
IN_F = 4096
OUT_F = 11008
BATCH = 8
SHARD = OUT_F // NCORES          # 1376 output features per core
P = 128
KT = IN_F // P                   # 32 k-tiles of 128
# Stationary operand columns: x_hi at 0..7, x_lo at 32..39 (zeros between).
LO_OFF = 32
M = LO_OFF + BATCH               # 40
# n-major column chunks (PSUM bank = 512 fp32); last is narrowest so the
# exposed tail epilogue is minimal.
CHUNKS = [(0, 512), (512, 512), (1024, 352)]
# k-tile grouping per chunk: list of group sizes summing to KT
K_GROUPS = {512: [2] * 16, 352: [4] * 7 + [2, 1, 1]}

_CACHED_NC = {}


def _build_bass(reps=1):
    """Build the Bass module. reps>1 emits the body that many times with a
    full barrier between reps — used only for slope-timing benchmarks."""
    import concourse.bacc as bacc
    import concourse.mybir as mybir
    import concourse.tile as tile

    nc = bacc.Bacc("TRN2", target_bir_lowering=False, debug=False)

    # Host-pretiled weight chunks: wt{j}[t*P + p, n] = W_c[n0 + n, t*128 + p]
    wts = [
        nc.dram_tensor(f"wt{j}", [KT * P, w], mybir.dt.float16, kind="ExternalInput")
        for j, (n0, w) in enumerate(CHUNKS)
    ]
    # Host-pretiled x (hi/lo split): xt[p, t*M + m] = xw[t*128 + p, m]
    xt = nc.dram_tensor("xt", [P, KT * M], mybir.dt.float16, kind="ExternalInput")
    # bias hi/lo rows: b2[0, n] = bias_hi, b2[1, n] = bias_lo
    b2 = nc.dram_tensor("b2", [2, SHARD], mybir.dt.float16, kind="ExternalInput")
    out = nc.dram_tensor("out", [BATCH, SHARD], mybir.dt.float32, kind="ExternalOutput")

    with tile.TileContext(nc) as tc:
        with (
            tc.tile_pool(name="consts", bufs=1) as cpool,
            tc.tile_pool(name="wtiles", bufs=8) as wpool,
            tc.tile_pool(name="acc", bufs=len(CHUNKS), space="PSUM") as ppool,
            tc.tile_pool(name="outp", bufs=1) as opool,
        ):
            xt_sb = cpool.tile([P, KT * M], mybir.dt.float16)
            nc.sync.dma_start(out=xt_sb[:], in_=xt[:])
            b2_sb = cpool.tile([2, SHARD], mybir.dt.float16)
            nc.sync.dma_start(out=b2_sb[:], in_=b2[:])
            # ones[k, m] = 1 for m < BATCH else 0: adds (bias_hi + bias_lo)
            # into the hi half of the accumulator only.
            ones_sb = cpool.tile([2, M], mybir.dt.float16)
            nc.any.memset(ones_sb[:, 0:BATCH], 1.0)
            nc.any.memset(ones_sb[:, BATCH:M], 0.0)

            out_sb = opool.tile([BATCH, SHARD], mybir.dt.float32)
            lo_sb = opool.tile([BATCH, SHARD], mybir.dt.float32)

            for rep in range(reps):
                if rep:
                    tc.strict_bb_all_engine_barrier()
                _emit_body(nc, tc, wpool, ppool, wts, xt_sb, b2_sb, ones_sb, out_sb, lo_sb, out)

    nc.compile()
    return nc


def _emit_body(nc, tc, wpool, ppool, wts, xt_sb, b2_sb, ones_sb, out_sb, lo_sb, out):
    import concourse.mybir as mybir

    if True:
            for j, (n0, w) in enumerate(CHUNKS):
                psum = ppool.tile([M, w], mybir.dt.float32, tag="acc")
                nc.tensor.matmul(
                    psum[:],
                    ones_sb[:],
                    b2_sb[:, n0 : n0 + w],
                    start=True,
                    stop=False,
                )
                t = 0
                for g in K_GROUPS[w]:
                    wtile = wpool.tile([P, g, w], mybir.dt.float16, tag=f"w{g}_{w}")
                    nc.sync.dma_start(
                        out=wtile[:],
                        in_=wts[j][t * P : (t + g) * P, :].rearrange(
                            "(t p) n -> p t n", p=P
                        ),
                    )
                    for ti in range(g):
                        ktile = t + ti
                        nc.tensor.matmul(
                            psum[:],
                            xt_sb[:, ktile * M : (ktile + 1) * M],
                            wtile[:, ti, :],
                            start=False,
                            stop=ktile == KT - 1,
                        )
                    t += g

                # Chunk epilogue: combine hi+lo on DVE (TensorTensor may read
                # only one PSUM operand: stage lo through SBUF first), store.
                # For all but the last chunk this hides under the next
                # chunk's weight stream.
                nc.vector.tensor_copy(
                    out=lo_sb[:, n0 : n0 + w],
                    in_=psum[LO_OFF : LO_OFF + BATCH, :],
                )
                nc.vector.tensor_add(
                    out=out_sb[:, n0 : n0 + w],
                    in0=psum[0:BATCH, :],
                    in1=lo_sb[:, n0 : n0 + w],
                )
                # scalar (ACT) HWDGE queue: keeps the output store off the
                # sync-engine FIFO so it can't head-of-line-block the next
                # chunk's weight DMAs.
                nc.scalar.dma_start(out=out[:, n0 : n0 + w], in_=out_sb[:, n0 : n0 + w])


def _get_nc(reps=1):
    if reps not in _CACHED_NC:
        _CACHED_NC[reps] = _build_bass(reps)
    return _CACHED_NC[reps]


def _prepare_inputs(x, weight_fp16, bias):
    x32 = np.asarray(x, dtype=np.float32)
    x_hi = x32.astype(np.float16)
    x_lo = (x32 - x_hi.astype(np.float32)).astype(np.float16)
    xw = np.zeros((IN_F, M), dtype=np.float16)
    xw[:, 0:BATCH] = x_hi.T
    xw[:, LO_OFF : LO_OFF + BATCH] = x_lo.T
    xt = np.ascontiguousarray(
        xw.reshape(KT, P, M).transpose(1, 0, 2)
    ).reshape(P, KT * M)

    w = np.asarray(weight_fp16)
    assert w.dtype == np.float16 and w.shape == (OUT_F, IN_F)
    # wt{j}[c][t*P + p, n] = W[c*SHARD + n0 + n, t*128 + p]
    wt_chunks = []
    for n0, cw in CHUNKS:
        # [c, n, t, p] -> [c, t, p, n]
        blk = w.reshape(NCORES, SHARD, KT, P)[:, n0 : n0 + cw]
        wt_chunks.append(
            np.ascontiguousarray(blk.transpose(0, 2, 3, 1)).reshape(NCORES, KT * P, cw)
        )

    b32 = np.asarray(bias, dtype=np.float32)
    b_hi = b32.astype(np.float16)
    b_lo = (b32 - b_hi.astype(np.float32)).astype(np.float16)

    in_maps = []
    for c in range(NCORES):
        m = {
            "xt": xt,
            "b2": np.stack(
                [b_hi[c * SHARD : (c + 1) * SHARD], b_lo[c * SHARD : (c + 1) * SHARD]]
            ),
        }
        for j in range(len(CHUNKS)):
            m[f"wt{j}"] = wt_chunks[j][c]
        in_maps.append(m)
    return in_maps


def _run(in_maps, **kwargs):
    from concourse.bass_utils import run_bass_kernel_spmd

    return run_bass_kernel_spmd(_get_nc(), in_maps, core_ids=list(range(NCORES)), **kwargs)


def kernel(x, weight_fp16, bias):
    res = _run(_prepare_inputs(x, weight_fp16, bias))
    out = np.concatenate([res.results[c]["out"] for c in range(NCORES)], axis=1)
    return np.ascontiguousarray(out, dtype=np.float32)


# revision 17
# speedup vs baseline: 1.4311x; 1.4128x over previous
"""Tensor-parallel compressed-linear (fp16 weights, fp32 IO) for 8 trn2 cores.

out[8, 11008] = x[8, 4096] @ W.T + bias    (W stored fp16, math in fp32)

Strategy (per spec sharding hint): shard W rows (out_features) across the 8
cores, replicate x, keep per-core output sharded along the feature dim and
concatenate on the host.

Per-core kernel: out_c[8, 1376] = x @ W_c.T + bias_c, memory-bound on the
11.27 MB fp16 weight shard (~31.5 us roofline at ~358 GB/s per-core HBM BW).

Device-side design:
  - The PE matmul contracts over the partition dim, so weights are fed as
    W.T tiles [k=128 partitions, n free]. We pre-transpose W on the host
    into a partition-major layout so every weight DMA is large and fully
    contiguous (no on-chip transpose, no strided descriptors).
  - x is fp32 but the PE streams fp16. We split x = x_hi + x_lo (two fp16
    halves) as columns of the stationary operand: one PE pass over the
    weight stream computes both, a cheap DVE add recombines them ->
    ~fp32 accuracy at zero extra weight traffic. The lo half sits at
    column/partition 32 because compute-engine operand partition offsets
    must be 32-aligned.
  - bias is folded into the PSUM accumulation via a K=2 matmul with
    (bias_hi, bias_lo) fp16 rows.
  - Weights stream n-major in 3 column-chunks (512/512/352 wide). Each
    chunk covers all of k, so its accumulator closes while the next chunk
    is still streaming and its epilogue (PSUM combine + store) hides under
    the stream; only the last, narrowest chunk's epilogue is exposed.
  - Each chunk streams as ~256-352 KB DMAs (k-pairs/quads): big enough
    that descriptor generation (~0.6 us/DMA) stays ahead of the wire,
    small enough that the PE tracks the stream closely.
"""

import numpy as np

NCORES = 8
IN_F = 4096
OUT_F = 11008
BATCH = 8
SHARD = OUT_F // NCORES          # 1376 output features per core
P = 128
KT = IN_F // P                   # 32 k-tiles of 128
# Stationary operand columns: x_hi at 0..7, x_lo at 32..39 (zeros between).
LO_OFF = 32
M = LO_OFF + BATCH               # 40
# n-major column chunks (PSUM bank = 512 fp32); last is narrowest so the
# exposed tail epilogue is minimal.
CHUNKS = [(0, 512), (512, 512), (1024, 352)]
# k-tile grouping per chunk: list of group sizes summing to KT
K_GROUPS = {512: [2] * 16, 352: [4] * 7 + [2, 1, 1]}

_CACHED_NC = {}
# bench knobs (mutated by bench harness only)
DMA_ONLY = False


def _build_bass(reps=1):
    """Build the Bass module. reps>1 emits the body that many times with a
    full barrier between reps — used only for slope-timing benchmarks."""
    import concourse.bacc as bacc
    import concourse.mybir as mybir
    import concourse.tile as tile

    nc = bacc.Bacc("TRN2", target_bir_lowering=False, debug=False)

    # Host-pretiled weight chunks: wt{j}[t*P + p, n] = W_c[n0 + n, t*128 + p]
    wts = [
        nc.dram_tensor(f"wt{j}", [KT * P, w], mybir.dt.float16, kind="ExternalInput")
        for j, (n0, w) in enumerate(CHUNKS)
    ]
    # Host-pretiled x (hi/lo split): xt[p, t*M + m] = xw[t*128 + p, m]
    xt = nc.dram_tensor("xt", [P, KT * M], mybir.dt.float16, kind="ExternalInput")
    # bias hi/lo rows: b2[0, n] = bias_hi, b2[1, n] = bias_lo
    b2 = nc.dram_tensor("b2", [2, SHARD], mybir.dt.float16, kind="ExternalInput")
    out = nc.dram_tensor("out", [BATCH, SHARD], mybir.dt.float32, kind="ExternalOutput")

    with tile.TileContext(nc) as tc:
        with (
            tc.tile_pool(name="consts", bufs=1) as cpool,
            tc.tile_pool(name="wtiles", bufs=8) as wpool,
            tc.tile_pool(name="acc", bufs=len(CHUNKS), space="PSUM") as ppool,
            tc.tile_pool(name="outp", bufs=1) as opool,
        ):
            xt_sb = cpool.tile([P, KT * M], mybir.dt.float16)
            nc.sync.dma_start(out=xt_sb[:], in_=xt[:])
            b2_sb = cpool.tile([2, SHARD], mybir.dt.float16)
            nc.sync.dma_start(out=b2_sb[:], in_=b2[:])
            # ones[k, m] = 1 for m < BATCH else 0: adds (bias_hi + bias_lo)
            # into the hi half of the accumulator only.
            ones_sb = cpool.tile([2, M], mybir.dt.float16)
            nc.any.memset(ones_sb[:, 0:BATCH], 1.0)
            nc.any.memset(ones_sb[:, BATCH:M], 0.0)

            out_sb = opool.tile([BATCH, SHARD], mybir.dt.float32)
            lo_sb = opool.tile([BATCH, SHARD], mybir.dt.float32)

            for rep in range(reps):
                if rep:
                    tc.strict_bb_all_engine_barrier()
                _emit_body(nc, tc, wpool, ppool, wts, xt_sb, b2_sb, ones_sb, out_sb, lo_sb, out)

    nc.compile()
    return nc


def _emit_body(nc, tc, wpool, ppool, wts, xt_sb, b2_sb, ones_sb, out_sb, lo_sb, out):
    import concourse.mybir as mybir

    if True:
            for j, (n0, w) in enumerate(CHUNKS):
                psum = ppool.tile([M, w], mybir.dt.float32, tag="acc")
                if not DMA_ONLY:
                    nc.tensor.matmul(
                        psum[:],
                        ones_sb[:],
                        b2_sb[:, n0 : n0 + w],
                        start=True,
                        stop=False,
                    )
                t = 0
                for g in K_GROUPS[w]:
                    wtile = wpool.tile([P, g, w], mybir.dt.float16, tag=f"w{g}_{w}")
                    nc.sync.dma_start(
                        out=wtile[:],
                        in_=wts[j][t * P : (t + g) * P, :].rearrange(
                            "(t p) n -> p t n", p=P
                        ),
                    )
                    if not DMA_ONLY:
                        for ti in range(g):
                            ktile = t + ti
                            nc.tensor.matmul(
                                psum[:],
                                xt_sb[:, ktile * M : (ktile + 1) * M],
                                wtile[:, ti, :],
                                start=False,
                                stop=ktile == KT - 1,
                            )
                    t += g
                if DMA_ONLY:
                    continue

                # Chunk epilogue: combine hi+lo on DVE (TensorTensor may read
                # only one PSUM operand: stage lo through SBUF first), store.
                # For all but the last chunk this hides under the next
                # chunk's weight stream.
                nc.vector.tensor_copy(
                    out=lo_sb[:, n0 : n0 + w],
                    in_=psum[LO_OFF : LO_OFF + BATCH, :],
                )
                nc.vector.tensor_add(
                    out=out_sb[:, n0 : n0 + w],
                    in0=psum[0:BATCH, :],
                    in1=lo_sb[:, n0 : n0 + w],
                )
                # scalar (ACT) HWDGE queue: keeps the output store off the
                # sync-engine FIFO so it can't head-of-line-block the next
                # chunk's weight DMAs.
                nc.scalar.dma_start(out=out[:, n0 : n0 + w], in_=out_sb[:, n0 : n0 + w])


def _get_nc(reps=1):
    if reps not in _CACHED_NC:
        _CACHED_NC[reps] = _build_bass(reps)
    return _CACHED_NC[reps]


def _prepare_inputs(x, weight_fp16, bias):
    x32 = np.asarray(x, dtype=np.float32)
    x_hi = x32.astype(np.float16)
    x_lo = (x32 - x_hi.astype(np.float32)).astype(np.float16)
    xw = np.zeros((IN_F, M), dtype=np.float16)
    xw[:, 0:BATCH] = x_hi.T
    xw[:, LO_OFF : LO_OFF + BATCH] = x_lo.T
    xt = np.ascontiguousarray(
        xw.reshape(KT, P, M).transpose(1, 0, 2)
    ).reshape(P, KT * M)

    w = np.asarray(weight_fp16)
    assert w.dtype == np.float16 and w.shape == (OUT_F, IN_F)
    # wt{j}[c][t*P + p, n] = W[c*SHARD + n0 + n, t*128 + p]
    wt_chunks = []
    for n0, cw in CHUNKS:
        # [c, n, t, p] -> [c, t, p, n]
        blk = w.reshape(NCORES, SHARD, KT, P)[:, n0 : n0 + cw]
        wt_chunks.append(
            np.ascontiguousarray(blk.transpose(0, 2, 3, 1)).reshape(NCORES, KT * P, cw)
        )

    b32 = np.asarray(bias, dtype=np.float32)
    b_hi = b32.astype(np.float16)
    b_lo = (b32 - b_hi.astype(np.float32)).astype(np.float16)

    in_maps = []
    for c in range(NCORES):
        m = {
            "xt": xt,
            "b2": np.stack(
                [b_hi[c * SHARD : (c + 1) * SHARD], b_lo[c * SHARD : (c + 1) * SHARD]]
            ),
        }
        for j in range(len(CHUNKS)):
            m[f"wt{j}"] = wt_chunks[j][c]
        in_maps.append(m)
    return in_maps


def _run(in_maps, **kwargs):
    from concourse.bass_utils import run_bass_kernel_spmd

    return run_bass_kernel_spmd(_get_nc(), in_maps, core_ids=list(range(NCORES)), **kwargs)


def kernel(x, weight_fp16, bias):
    res = _run(_prepare_inputs(x, weight_fp16, bias))
    out = np.concatenate([res.results[c]["out"] for c in range(NCORES)], axis=1)
    return np.ascontiguousarray(out, dtype=np.float32)


# revision 20
# speedup vs baseline: 1.5884x; 1.1099x over previous
"""Tensor-parallel compressed-linear (fp16 weights, fp32 IO) for 8 trn2 cores.

out[8, 11008] = x[8, 4096] @ W.T + bias    (W stored fp16, math in fp32)

Strategy (per spec sharding hint): shard W rows (out_features) across the 8
cores, replicate x, keep per-core output sharded along the feature dim and
concatenate on the host.

Per-core kernel: out_c[8, 1376] = x @ W_c.T + bias_c, memory-bound on the
11.27 MB fp16 weight shard (~31.5 us roofline at ~358 GB/s per-core HBM BW).

Device-side design:
  - The PE matmul contracts over the partition dim, so weights are fed as
    W.T tiles [k=128 partitions, n free]. We pre-transpose W on the host
    into a partition-major layout so every weight DMA is large and fully
    contiguous (no on-chip transpose, no strided descriptors).
  - x is fp32 but the PE streams fp16. We split x = x_hi + x_lo (two fp16
    halves) as columns of the stationary operand: one PE pass over the
    weight stream computes both, a cheap DVE add recombines them ->
    ~fp32 accuracy at zero extra weight traffic. The lo half sits at
    column/partition 32 because compute-engine operand partition offsets
    must be 32-aligned.
  - bias is folded into the PSUM accumulation via a K=2 matmul with
    (bias_hi, bias_lo) fp16 rows.
  - Weights stream n-major in 3 column-chunks (512/512/352 wide). Each
    chunk covers all of k, so its accumulator closes while the next chunk
    is still streaming and its epilogue (PSUM combine + store) hides under
    the stream; only the last, narrowest chunk's epilogue is exposed.
  - Each chunk streams as ~256-352 KB DMAs (k-pairs/quads): big enough
    that descriptor generation (~0.6 us/DMA) stays ahead of the wire,
    small enough that the PE tracks the stream closely.
"""

import numpy as np

NCORES = 8
IN_F = 4096
OUT_F = 11008
BATCH = 8
SHARD = OUT_F // NCORES          # 1376 output features per core
P = 128
KT = IN_F // P                   # 32 k-tiles of 128
# Stationary operand columns: x_hi at 0..7, x_lo at 32..39 (zeros between).
LO_OFF = 32
M = LO_OFF + BATCH               # 40
# n-major column chunks (PSUM bank = 512 fp32); last is narrowest so the
# exposed tail epilogue is minimal.
CHUNKS = [(0, 512), (512, 512), (1024, 352)]
# k-tile grouping per chunk: list of group sizes summing to KT
K_GROUPS = {512: [2] * 16, 352: [4] * 7 + [2, 1, 1]}

_CACHED_NC = {}
# bench knobs (mutated by bench harness only)
DMA_ONLY = False


def _build_bass(reps=1):
    """Build the Bass module. reps>1 emits the body that many times with a
    full barrier between reps — used only for slope-timing benchmarks."""
    import concourse.bacc as bacc
    import concourse.mybir as mybir
    import concourse.tile as tile

    nc = bacc.Bacc("TRN2", target_bir_lowering=False, debug=False)

    # Host-pretiled weight chunks: wt{j}[t*P + p, n] = W_c[n0 + n, t*128 + p]
    wts = [
        nc.dram_tensor(f"wt{j}", [KT * P, w], mybir.dt.float16, kind="ExternalInput")
        for j, (n0, w) in enumerate(CHUNKS)
    ]
    # Host-pretiled x (hi/lo split): xt[p, t*M + m] = xw[t*128 + p, m]
    xt = nc.dram_tensor("xt", [P, KT * M], mybir.dt.float16, kind="ExternalInput")
    # bias hi/lo rows: b2[0, n] = bias_hi, b2[1, n] = bias_lo
    b2 = nc.dram_tensor("b2", [2, SHARD], mybir.dt.float16, kind="ExternalInput")
    out = nc.dram_tensor("out", [BATCH, SHARD], mybir.dt.float32, kind="ExternalOutput")

    with tile.TileContext(nc) as tc:
        with (
            tc.tile_pool(name="consts", bufs=1) as cpool,
            # per-tag bufs below make every weight tile of one pass resident:
            # a WAR wait on a reused slot would head-of-line-block the
            # in-order sync sequencer and stall the whole DMA stream.
            tc.tile_pool(name="wtiles", bufs=1) as wpool,
            tc.tile_pool(name="acc", bufs=len(CHUNKS), space="PSUM") as ppool,
            tc.tile_pool(name="outp", bufs=1) as opool,
        ):
            xt_sb = cpool.tile([P, KT * M], mybir.dt.float16)
            nc.sync.dma_start(out=xt_sb[:], in_=xt[:])
            b2_sb = cpool.tile([2, SHARD], mybir.dt.float16)
            nc.sync.dma_start(out=b2_sb[:], in_=b2[:])
            # ones[k, m] = 1 for m < BATCH else 0: adds (bias_hi + bias_lo)
            # into the hi half of the accumulator only.
            ones_sb = cpool.tile([2, M], mybir.dt.float16)
            nc.any.memset(ones_sb[:, 0:BATCH], 1.0)
            nc.any.memset(ones_sb[:, BATCH:M], 0.0)

            out_sb = opool.tile([BATCH, SHARD], mybir.dt.float32)
            lo_sb = opool.tile([BATCH, SHARD], mybir.dt.float32)

            for rep in range(reps):
                if rep:
                    tc.strict_bb_all_engine_barrier()
                _emit_body(nc, tc, wpool, ppool, wts, xt_sb, b2_sb, ones_sb, out_sb, lo_sb, out)

    nc.compile()
    return nc


def _emit_body(nc, tc, wpool, ppool, wts, xt_sb, b2_sb, ones_sb, out_sb, lo_sb, out):
    import concourse.mybir as mybir

    if True:
            for j, (n0, w) in enumerate(CHUNKS):
                psum = ppool.tile([M, w], mybir.dt.float32, tag="acc")
                if not DMA_ONLY:
                    nc.tensor.matmul(
                        psum[:],
                        ones_sb[:],
                        b2_sb[:, n0 : n0 + w],
                        start=True,
                        stop=False,
                    )
                t = 0
                n_chunks_w = sum(1 for _, cw in CHUNKS if cw == w)
                for g in K_GROUPS[w]:
                    wtile = wpool.tile(
                        [P, g, w],
                        mybir.dt.float16,
                        tag=f"w{g}_{w}",
                        bufs=K_GROUPS[w].count(g) * n_chunks_w,
                    )
                    nc.sync.dma_start(
                        out=wtile[:],
                        in_=wts[j][t * P : (t + g) * P, :].rearrange(
                            "(t p) n -> p t n", p=P
                        ),
                    )
                    if not DMA_ONLY:
                        for ti in range(g):
                            ktile = t + ti
                            nc.tensor.matmul(
                                psum[:],
                                xt_sb[:, ktile * M : (ktile + 1) * M],
                                wtile[:, ti, :],
                                start=False,
                                stop=ktile == KT - 1,
                            )
                    t += g
                if DMA_ONLY:
                    continue

                # Chunk epilogue: combine hi+lo on DVE (TensorTensor may read
                # only one PSUM operand: stage lo through SBUF first), store.
                # For all but the last chunk this hides under the next
                # chunk's weight stream.
                nc.vector.tensor_copy(
                    out=lo_sb[:, n0 : n0 + w],
                    in_=psum[LO_OFF : LO_OFF + BATCH, :],
                )
                nc.vector.tensor_add(
                    out=out_sb[:, n0 : n0 + w],
                    in0=psum[0:BATCH, :],
                    in1=lo_sb[:, n0 : n0 + w],
                )
                # scalar (ACT) HWDGE queue: keeps the output store off the
                # sync-engine FIFO so it can't head-of-line-block the next
                # chunk's weight DMAs.
                nc.scalar.dma_start(out=out[:, n0 : n0 + w], in_=out_sb[:, n0 : n0 + w])


def _get_nc(reps=1):
    if reps not in _CACHED_NC:
        _CACHED_NC[reps] = _build_bass(reps)
    return _CACHED_NC[reps]


def _prepare_inputs(x, weight_fp16, bias):
    x32 = np.asarray(x, dtype=np.float32)
    x_hi = x32.astype(np.float16)
    x_lo = (x32 - x_hi.astype(np.float32)).astype(np.float16)
    xw = np.zeros((IN_F, M), dtype=np.float16)
    xw[:, 0:BATCH] = x_hi.T
    xw[:, LO_OFF : LO_OFF + BATCH] = x_lo.T
    xt = np.ascontiguousarray(
        xw.reshape(KT, P, M).transpose(1, 0, 2)
    ).reshape(P, KT * M)

    w = np.asarray(weight_fp16)
    assert w.dtype == np.float16 and w.shape == (OUT_F, IN_F)
    # wt{j}[c][t*P + p, n] = W[c*SHARD + n0 + n, t*128 + p]
    wt_chunks = []
    for n0, cw in CHUNKS:
        # [c, n, t, p] -> [c, t, p, n]
        blk = w.reshape(NCORES, SHARD, KT, P)[:, n0 : n0 + cw]
        wt_chunks.append(
            np.ascontiguousarray(blk.transpose(0, 2, 3, 1)).reshape(NCORES, KT * P, cw)
        )

    b32 = np.asarray(bias, dtype=np.float32)
    b_hi = b32.astype(np.float16)
    b_lo = (b32 - b_hi.astype(np.float32)).astype(np.float16)

    in_maps = []
    for c in range(NCORES):
        m = {
            "xt": xt,
            "b2": np.stack(
                [b_hi[c * SHARD : (c + 1) * SHARD], b_lo[c * SHARD : (c + 1) * SHARD]]
            ),
        }
        for j in range(len(CHUNKS)):
            m[f"wt{j}"] = wt_chunks[j][c]
        in_maps.append(m)
    return in_maps


def _run(in_maps, **kwargs):
    from concourse.bass_utils import run_bass_kernel_spmd

    return run_bass_kernel_spmd(_get_nc(), in_maps, core_ids=list(range(NCORES)), **kwargs)


def kernel(x, weight_fp16, bias):
    res = _run(_prepare_inputs(x, weight_fp16, bias))
    out = np.concatenate([res.results[c]["out"] for c in range(NCORES)], axis=1)
    return np.ascontiguousarray(out, dtype=np.float32)
